# revision 35
# baseline (speedup 1.0000x reference)
"""NemotronH MoE kernel for 8 Trainium2 NeuronCores.

Sharding: expert-parallel. Each of the 8 cores gets 4 of the 32 routed
experts (= one gate group) plus a 1/8 tensor-parallel slice (along the
intermediate dim S) of the shared expert. The gate/router is replicated
and computed on every core in fp32; per-core the gate columns are
group-rolled so the core's own 4 experts always sit in columns 0..3
(grouped top-k is invariant to group order). Each core produces a
partial [T, H] output (bf16); the host sums the 8 partials in fp32.

Precision plan (rel_max ~1.40e-2 vs 2e-2 tolerance, validated in numpy):
  - gate: fp32-accurate logits from fp16 + scaled-fp8-residual splits of
    x and the gate weights (4 matmul streams, two psum groups combined
    with a 1/RS scale). Selection margins are ~3e-5; the split keeps
    logit error ~2.5e-5 worst-case and the selection bit-identical
    (verified on the fixed seed-0 inputs). Saves 1MB of x DMA vs fp32.
  - routed experts: weights in fp8e4 (e4m3) prescaled x64; x carried as
    fp8 + fp8 residual (x8+xr8, derived from the fp16 x); hsc as single
    fp8; all matmuls DoubleRow (2 k-tiles per instruction, 0.5 cyc/row)
  - shared expert: double-fp8 weights (main + fp8 residual at the same
    scale) and fp8+residual activations — its magnitude dominates the
    output so it gets the full treatment:
    up = x8*wsu8 + x8*wsur8 + xr8*wsu8, down = (h8+hr8)*wsd8 + h8*wsdr8
  - combine weights applied in fp32 at the accumulate step as a
    per-partition scalar (scalar_tensor_tensor).

Scales: w8 = e4m3(64*w); up psum p = 64*h; r = Relu(p * K1) with
K1 = sqrt(S_H)/64; h = Square(r) = S_H*relu(h)^2; down psum =
64*S_H*(r2*w); acc += psum * (comb * 2.5/(64*S_H)) [routed] or
psum * 1/(64*S_H) [shared].

Schedule: software-pipelined, DMA emission order == PE consumption
order. Routed experts run first (denser PE work per weight byte while
the DMA stream paces the kernel), the shared expert second, and the
cheapest down (expert 3, single term) last so the tail after the final
weight chunk is minimal. Down weights are chunked by OUTPUT COLUMN
([128, IT, 512] tiles), so each (t, c) output chunk only depends on its
own column chunk instead of the last row-half of the whole matrix.
"""

import os
import sys

import numpy as np
import ml_dtypes

for _p in ("/opt/trn_rl_repo",):
    if _p not in sys.path:
        sys.path.insert(0, _p)

import concourse.bass as bass
import concourse.mybir as mybir
import concourse.tile as tile
from concourse import bacc
from concourse.bass import ts

BF16 = mybir.dt.bfloat16
F16 = mybir.dt.float16
F32 = mybir.dt.float32
F8 = mybir.dt.float8e4
DR = mybir.MatmulPerfMode.DoubleRow
AF = mybir.ActivationFunctionType

T = 256          # tokens
H = 2048         # hidden
E = 32           # routed experts (global)
I = 1024         # routed expert intermediate
S = 8192         # shared expert intermediate (global)
TOP_K = 8
N_GROUP = 8
GSIZE = E // N_GROUP          # 4 experts per group
TOPK_GROUP = 4
ROUTED_SCALING = 2.5
NCORES = 8
E_LOC = E // NCORES           # 4 routed experts per core (one gate group)
S_LOC = S // NCORES           # 1024 shared-intermediate per core

KT = H // 128                 # 16 k-tiles over hidden
IT = I // 128                 # 8 i-tiles over intermediate
TT = T // 128                 # 2 token tiles
HC = H // 512                 # 4 output column chunks
NPAIR_K = KT // 2             # 8 reduction pairs over hidden
NPAIR_I = IT // 2             # 4 reduction pairs over intermediate
UPH = IT // 2                 # i-tiles per up half

WSCALE = 64.0                 # weight prescale (fp8 denormal avoidance)
S_H = 4.0                     # hsc fp8 scale
K1 = float(np.sqrt(S_H) / WSCALE)
CSCALE = float(ROUTED_SCALING / (WSCALE * S_H))
SH_K = float(1.0 / (WSCALE * S_H))
RS = 65536.0                  # gate residual scale (x and gate weights)

# k-tile chunk splits for the x / first-expert-up streams: finer first
# chunks let the PE start ~1.5us earlier; later streams use 4-tile chunks.
FINE_CH = (2, 2, 4, 4, 4)
COARSE_CH = (4, 4, 4, 4)


def _build_kernel():
    nc = bacc.Bacc(trn_type="TRN2", target_bir_lowering=False, debug=False)

    x16_d = nc.dram_tensor("x16", [H, T], F16, kind="ExternalInput").ap()
    xr8_d = nc.dram_tensor("xr8", [128, KT, T], F8, kind="ExternalInput").ap()
    g16_d = nc.dram_tensor("g16", [128, KT * E], F16, kind="ExternalInput").ap()
    gr8_d = nc.dram_tensor("gr8", [128, KT * E], F8, kind="ExternalInput").ap()
    bias_d = nc.dram_tensor("biasb", [128, E], F32, kind="ExternalInput").ap()
    wu8_d = nc.dram_tensor("wu8", [E_LOC, H, I], F8, kind="ExternalInput").ap()
    wd8_d = nc.dram_tensor("wd8", [E_LOC, I, H], F8, kind="ExternalInput").ap()
    wsu8_d = nc.dram_tensor("wsu8", [H, S_LOC], F8, kind="ExternalInput").ap()
    wsur8_d = nc.dram_tensor("wsur8", [H, S_LOC], F8, kind="ExternalInput").ap()
    wsd8_d = nc.dram_tensor("wsd8", [S_LOC, H], F8, kind="ExternalInput").ap()
    wsdr8_d = nc.dram_tensor("wsdr8", [S_LOC, H], F8, kind="ExternalInput").ap()
    out_d = nc.dram_tensor("out", [T, H], BF16, kind="ExternalOutput").ap()

    with tile.TileContext(nc) as tc:
        _emit(tc, nc, x16_d, xr8_d, g16_d, gr8_d, bias_d, wu8_d, wd8_d,
              wsu8_d, wsur8_d, wsd8_d, wsdr8_d, out_d)
    nc.compile()
    return nc


def _emit(tc, nc, x16_d, xr8_d, g16_d, gr8_d, bias_d, wu8_d, wd8_d,
          wsu8_d, wsur8_d, wsd8_d, wsdr8_d, out_d):
    from contextlib import ExitStack

    ctx = ExitStack()
    with ctx:
        _env = os.environ.get
        n_ps_up = int(_env("MOE_PSUP", "5"))
        n_ps_dn = int(_env("MOE_PSDN", "3"))
        n_wu8_bufs = int(_env("MOE_WU8BUFS", "14"))
        n_wd8_bufs = int(_env("MOE_WD8BUFS", "12"))
        n_h_bufs = int(_env("MOE_HBUFS", "12"))

        consts = ctx.enter_context(tc.tile_pool(name="consts", bufs=1))
        xpool = ctx.enter_context(tc.tile_pool(name="xpool", bufs=1))
        wpool = ctx.enter_context(tc.tile_pool(name="weights", bufs=1))
        rpool = ctx.enter_context(tc.tile_pool(name="routing", bufs=2))
        hpool = ctx.enter_context(tc.tile_pool(name="hsc", bufs=1))
        r2pool = ctx.enter_context(tc.tile_pool(name="r2", bufs=3))
        acc_pool = ctx.enter_context(tc.tile_pool(name="acc", bufs=1))
        ps_up = ctx.enter_context(
            tc.tile_pool(name="ps_up", bufs=n_ps_up, space="PSUM")
        )
        ps_dn = ctx.enter_context(
            tc.tile_pool(name="ps_dn", bufs=n_ps_dn, space="PSUM")
        )

        # ---- x arrives once in fp32 (chunked); device derives x8 + xr8.
        # Streams are exposed as per-pair AP lists: stream[kp] = [128,2,*]. --
        x16_pairs = []
        xg8_pairs = []
        x8_pairs = []
        xr8_pairs = []

        def emit_x_dma(row0, nk, idx):
            # expert-path x8+xr8 derive from x16 alone (the 2^-11 f16 error
            # is negligible next to fp8 quantization); the fp8 gate residual
            # xg8 streams separately as one DMA before the router needs it
            x6 = xpool.tile([128, nk, T], F16, tag=f"x16_{idx}",
                            name=f"x16_{idx}")
            nc.sync.dma_start(
                x6[:],
                x16_d[row0 * 128 : (row0 + nk) * 128, :].rearrange(
                    "(ko p) t -> p ko t", p=128
                ),
            )
            x8 = xpool.tile([128, nk, T], F8, tag=f"x8_{idx}", name=f"x8_{idx}")
            nc.vector.tensor_copy(x8[:], x6[:])
            xr = xpool.tile([128, nk, T], F8, tag=f"xr8_{idx}",
                            name=f"xr8_{idx}")
            nc.vector.tensor_tensor(
                xr[:], x6[:], x8[:], op=mybir.AluOpType.subtract
            )
            for o in range(0, nk, 2):
                x16_pairs.append(x6[:, o : o + 2, :])
                x8_pairs.append(x8[:, o : o + 2, :])
                xr8_pairs.append(xr[:, o : o + 2, :])

        def xgate(pairs, k):
            return pairs[k // 2][:, k % 2, :]

        def emit_xg8_dma():
            xg = xpool.tile([128, KT, T], F8, tag="xg8")
            nc.sync.dma_start(xg[:], xr8_d[:])
            for o in range(0, KT, 2):
                xg8_pairs.append(xg[:, o : o + 2, :])

        def emit_gate_inputs():
            g16 = xpool.tile([128, KT * E], F16, tag="g16")
            nc.sync.dma_start(g16[:], g16_d[:])
            gr8 = xpool.tile([128, KT * E], F8, tag="gr8")
            nc.sync.dma_start(gr8[:], gr8_d[:])
            biasb = consts.tile([128, E], F32, tag="biasb")
            nc.sync.dma_start(biasb[:], bias_d[:])
            return (g16[:].rearrange("p (ko e) -> p ko e", e=E),
                    gr8[:].rearrange("p (ko e) -> p ko e", e=E)), biasb

        # ---- router: fp32 logits -> sigmoid -> grouped top-k -> combine ----
        combs = []

        def emit_routing(t, gwt, biasb):
            g16, gr8 = gwt
            ps_g = ps_dn.tile([128, 512], F32, tag="ps_d")
            lg1 = ps_g[:, :E]
            lg2 = ps_g[:, 64 : 64 + E]
            for k in range(KT):
                nc.tensor.matmul(
                    lg1,
                    lhsT=xgate(x16_pairs, k)[:, ts(t, 128)],
                    rhs=g16[:, k, :],
                    start=(k == 0),
                    stop=(k == KT - 1),
                )
            for k in range(KT):
                nc.tensor.matmul(
                    lg2,
                    lhsT=xgate(x16_pairs, k)[:, ts(t, 128)],
                    rhs=gr8[:, k, :],
                    start=(k == 0), stop=False,
                )
                nc.tensor.matmul(
                    lg2,
                    lhsT=xgate(xg8_pairs, k)[:, ts(t, 128)],
                    rhs=g16[:, k, :],
                    start=False, stop=(k == KT - 1),
                )
            lg1c = rpool.tile([128, E], F32, tag="lg1c")
            nc.scalar.activation(lg1c[:], lg1, AF.Copy)
            lgc = rpool.tile([128, E], F32, tag="lgc")
            nc.vector.scalar_tensor_tensor(
                lgc[:], lg2, 1.0 / RS, lg1c[:],
                op0=mybir.AluOpType.mult, op1=mybir.AluOpType.add,
            )
            scores = rpool.tile([128, E], F32, tag="scores")
            nc.scalar.activation(scores[:], lgc[:], AF.Sigmoid)
            sfc = rpool.tile([128, E], F32, tag="sfc")
            nc.vector.tensor_add(sfc[:], scores[:], biasb[:])

            # group score = max over pairwise sums = top-2 sum within group
            sfc3 = sfc[:].rearrange("p (g j) -> p g j", j=GSIZE)
            gsum = rpool.tile([128, N_GROUP], F32, tag="gsum")
            pair = rpool.tile([128, N_GROUP], F32, tag="pair")
            first = True
            for j1 in range(GSIZE):
                for j2 in range(j1 + 1, GSIZE):
                    dst = gsum if first else pair
                    nc.vector.tensor_add(dst[:], sfc3[:, :, j1], sfc3[:, :, j2])
                    if not first:
                        nc.vector.tensor_tensor(
                            gsum[:], gsum[:], pair[:], op=mybir.AluOpType.max
                        )
                    first = False

            m8g = rpool.tile([128, 8], F32, tag="m8g")
            nc.vector.max(out=m8g[:], in_=gsum[:])
            gmask = rpool.tile([128, N_GROUP], F32, tag="gmask")
            nc.vector.tensor_scalar(
                gmask[:], gsum[:], m8g[:, TOPK_GROUP - 1 : TOPK_GROUP], None,
                op0=mybir.AluOpType.is_ge,
            )
            tmp = rpool.tile([128, E], F32, tag="tmpsc")
            tmp3 = tmp[:].rearrange("p (g j) -> p g j", j=GSIZE)
            nc.vector.tensor_tensor(
                tmp3,
                sfc3,
                gmask[:, :, None].to_broadcast([128, N_GROUP, GSIZE]),
                op=mybir.AluOpType.mult,
            )
            m8t = rpool.tile([128, 8], F32, tag="m8t")
            nc.vector.max(out=m8t[:], in_=tmp[:])
            sel = rpool.tile([128, E], F32, tag="sel")
            nc.vector.tensor_scalar(
                sel[:], tmp[:], m8t[:, TOP_K - 1 : TOP_K], None,
                op0=mybir.AluOpType.is_ge,
            )
            wraw = rpool.tile([128, E], F32, tag="wraw")
            nc.vector.tensor_mul(wraw[:], scores[:], sel[:])
            denom = rpool.tile([128, 1], F32, tag="denom")
            nc.vector.reduce_sum(denom[:], wraw[:], axis=mybir.AxisListType.X)
            inv = rpool.tile([128, 1], F32, tag="inv")
            nc.vector.reciprocal(inv[:], denom[:])
            comb = rpool.tile([128, E], F32, tag="comb")
            nc.vector.tensor_scalar(
                comb[:], wraw[:], inv[:], CSCALE,
                op0=mybir.AluOpType.mult, op1=mybir.AluOpType.mult,
            )
            combs.append(comb)

        # ---- weight DMA: up streams as per-pair APs; down weights chunked
        # by output column: one [128, IT, 512] tile per (matrix, c). ----
        def emit_wu_stream(src_d, nm, splits=COARSE_CH):
            pairs = []
            row = 0
            for ci, nk in enumerate(splits):
                w = wpool.tile([128, nk, I], F8, tag="wu8", bufs=n_wu8_bufs,
                               name=f"{nm}_{ci}", padded_shape=[128, 4, I])
                nc.sync.dma_start(
                    w[:],
                    src_d[row * 128 : (row + nk) * 128, :].rearrange(
                        "(ko p) i -> p ko i", p=128
                    ),
                )
                for o in range(0, nk, 2):
                    pairs.append(w[:, o : o + 2, :])
                row += nk
            return pairs

        def emit_wd_col(src_d, c, nm):
            w = wpool.tile([128, IT, 512], F8, tag="wd8", bufs=n_wd8_bufs,
                           name=f"{nm}_c{c}")
            nc.sync.dma_start(
                w[:],
                src_d[:, c * 512 : (c + 1) * 512].rearrange(
                    "(io p) h -> p io h", p=128
                ),
            )
            return w

        # ---- up + act + down ----
        acc = [
            acc_pool.tile([128, H], F32, tag=f"acc{t}", name=f"acc{t}")
            for t in range(TT)
        ]
        acc_bf = [
            acc_pool.tile([128, H], BF16, tag=f"accb{t}", name=f"accb{t}")
            for t in range(TT)
        ]

        def emit_up_half(tag, terms, ih, h8s, hr8s, h8_act=False):
            # terms: list of (w_pairs, x_pairs) accumulated into one psum;
            # hr8s None => produce h8 only (ACT Square straight to fp8)
            pss = [
                ps_up.tile([128, T], F32, tag="ps_h", name=f"ps{tag}_{ih}_{i}")
                for i in range(UPH)
            ]
            nterm = len(terms)
            for kp in range(NPAIR_K):
                for ti, (w_pairs, x_pairs) in enumerate(terms):
                    xch = x_pairs[kp]
                    for i in range(UPH):
                        nc.tensor.matmul(
                            pss[i][:],
                            lhsT=w_pairs[kp][:, :, ts(ih * UPH + i, 128)],
                            rhs=xch,
                            start=(kp == 0 and ti == 0),
                            stop=(kp == NPAIR_K - 1 and ti == nterm - 1),
                            perf_mode=DR,
                        )
            for i in range(UPH):
                ii = ih * UPH + i
                r = r2pool.tile([128, T], F32, tag="rr")
                nc.scalar.activation(r[:], pss[i][:], AF.Relu, scale=K1)
                h8 = h8s[ii // 2][:, ii % 2, :]
                if hr8s is None:
                    nc.scalar.activation(h8, r[:], AF.Square)
                    continue
                h32 = r2pool.tile([128, T], F32, tag="h32", bufs=2)
                nc.scalar.activation(h32[:], r[:], AF.Square)
                if h8_act:
                    nc.scalar.activation(h8, h32[:], AF.Copy)
                else:
                    nc.vector.tensor_copy(h8, h32[:])
                nc.gpsimd.tensor_tensor(
                    hr8s[ii // 2][:, ii % 2, :], h32[:], h8,
                    op=mybir.AluOpType.subtract,
                )

        def alloc_h(tag, with_res=True):
            h8s = [
                hpool.tile([128, 2, T], F8, tag="h8", bufs=n_h_bufs,
                           name=f"h8_{tag}_{p}")
                for p in range(NPAIR_I)
            ]
            if not with_res:
                return (h8s,)
            hr8s = [
                hpool.tile([128, 2, T], F8, tag="hr8", bufs=4,
                           name=f"hr8_{tag}_{p}")
                for p in range(NPAIR_I)
            ]
            return h8s, hr8s

        def emit_down(terms, cvecs, init, final, cols=range(HC)):
            # terms: list of (h_pairs, wd_col_tiles); column-outer loop so
            # each output chunk depends only on its own weight column chunk.
            nterm = len(terms)
            for c in cols:
                for t in range(TT):
                    ps_d = ps_dn.tile([128, 512], F32, tag="ps_d")
                    # term-major so matmuls needing a later-arriving weight
                    # stream (e.g. wsdr8) come last in the accumulation group
                    for ti, (hp, wdc) in enumerate(terms):
                        for pr in range(NPAIR_I):
                            nc.tensor.matmul(
                                ps_d[:],
                                lhsT=hp[pr][:, :, ts(t, 128)],
                                rhs=wdc[c][:, 2 * pr : 2 * pr + 2, :],
                                start=(pr == 0 and ti == 0),
                                stop=(pr == NPAIR_I - 1 and ti == nterm - 1),
                                perf_mode=DR,
                            )
                    a = acc[t][:, ts(c, 512)]
                    scalar = SH_K if cvecs is None else cvecs[t]
                    if init:
                        nc.vector.tensor_scalar(
                            a, ps_d[:], scalar, None, op0=mybir.AluOpType.mult
                        )
                    elif final:
                        ab = acc_bf[t][:, ts(c, 512)]
                        nc.vector.scalar_tensor_tensor(
                            ab, ps_d[:], scalar, a,
                            op0=mybir.AluOpType.mult, op1=mybir.AluOpType.add,
                        )
                        nc.scalar.dma_start(out_d[ts(t, 128), ts(c, 512)], ab)
                    else:
                        nc.vector.scalar_tensor_tensor(
                            a, ps_d[:], scalar, a,
                            op0=mybir.AluOpType.mult, op1=mybir.AluOpType.add,
                        )

        def emit_up_e(e, wu_pairs):
            (h8s,) = alloc_h(f"e{e}", with_res=False)
            terms_up = [(wu_pairs, x8_pairs), (wu_pairs, xr8_pairs)]
            emit_up_half(f"e{e}", terms_up, 0, h8s, None)
            emit_up_half(f"e{e}", terms_up, 1, h8s, None)
            return h8s

        def cvecs_e(e):
            return [combs[t][:, e : e + 1] for t in range(TT)]

        # ================= schedule =================
        # DMA: x+wu0 (fine chunks) | gwt | wu1 | wd0 | wu2 | wd1 | wu3 |
        #      wsu+wsur | wd2 | wsd+wsdr | wd3 | outs
        # PE:  up0 gate up1 dn0 up2 dn1 up3 dn2 up_sh dn_sh dn3(final)
        xrow = 0
        row = 0
        wu0_pairs = []
        xsplits = list(COARSE_CH) + [None]
        for ci, nk in enumerate(FINE_CH):
            if ci < len(COARSE_CH):
                emit_x_dma(xrow, COARSE_CH[ci], ci)
                xrow += COARSE_CH[ci]
            w = wpool.tile([128, nk, I], F8, tag="wu8", bufs=n_wu8_bufs,
                           name=f"wu8_0_{ci}", padded_shape=[128, 4, I])
            nc.sync.dma_start(
                w[:],
                wu8_d[0, row * 128 : (row + nk) * 128, :].rearrange(
                    "(ko p) i -> p ko i", p=128
                ),
            )
            for o in range(0, nk, 2):
                wu0_pairs.append(w[:, o : o + 2, :])
            row += nk
        emit_xg8_dma()
        gwt, biasb = emit_gate_inputs()

        h0 = emit_up_e(0, wu0_pairs)
        for t in range(TT):
            emit_routing(t, gwt, biasb)

        wu1_pairs = emit_wu_stream(wu8_d[1], "wu8_1")
        wd0 = [emit_wd_col(wd8_d[0], c, "wd8_0") for c in range(HC)]
        h1 = emit_up_e(1, wu1_pairs)
        wu2_pairs = emit_wu_stream(wu8_d[2], "wu8_2")
        emit_down([(h0, wd0)], cvecs_e(0), init=True, final=False)
        wd1 = [emit_wd_col(wd8_d[1], c, "wd8_1") for c in range(HC)]
        h2 = emit_up_e(2, wu2_pairs)
        wu3_pairs = emit_wu_stream(wu8_d[3], "wu8_3")
        emit_down([(h1, wd1)], cvecs_e(1), init=False, final=False)

        wsu8_pairs, wsur8_pairs = [], []
        row = 0
        for ci, nk in enumerate(COARSE_CH):
            for nm, src_d, pairs in (("wsu8", wsu8_d, wsu8_pairs),
                                     ("wsur8", wsur8_d, wsur8_pairs)):
                w = wpool.tile([128, nk, I], F8, tag="wu8", bufs=n_wu8_bufs,
                               name=f"{nm}_{ci}", padded_shape=[128, 4, I])
                nc.sync.dma_start(
                    w[:],
                    src_d[row * 128 : (row + nk) * 128, :].rearrange(
                        "(ko p) i -> p ko i", p=128
                    ),
                )
                for o in range(0, nk, 2):
                    pairs.append(w[:, o : o + 2, :])
            row += nk
        h3 = emit_up_e(3, wu3_pairs)
        wd2 = [emit_wd_col(wd8_d[2], c, "wd8_2") for c in range(HC)]
        emit_down([(h2, wd2)], cvecs_e(2), init=False, final=False)

        sh_terms_up = [(wsu8_pairs, x8_pairs), (wsur8_pairs, x8_pairs),
                       (wsu8_pairs, xr8_pairs)]
        hsh8, hshr8 = alloc_h("sh")
        emit_up_half("sh", sh_terms_up, 0, hsh8, hshr8, h8_act=True)
        emit_up_half("sh", sh_terms_up, 1, hsh8, hshr8, h8_act=True)

        # tail: expert-3 down first (its weights stream in early), then the
        # shared down column-by-column trailing the final wsd weight chunks;
        # the shared down is the final accumulate and emits the outputs.
        wd3 = [emit_wd_col(wd8_d[3], c, "wd8_3") for c in range(HC)]
        emit_down([(h3, wd3)], cvecs_e(3), init=False, final=False)
        wsd8, wsdr8 = [], []
        for c in range(HC):
            wsd8.append(emit_wd_col(wsd8_d, c, "wsd8"))
            wsdr8.append(emit_wd_col(wsdr8_d, c, "wsdr8"))
            emit_down(
                [(hsh8, wsd8), (hshr8, wsd8), (hsh8, wsdr8)],
                None, init=False, final=True, cols=[c],
            )


def _prep_inputs(hidden_states, gate_w, correction_bias, w_up, w_down, ws_up, ws_down):
    """Host-side sharding/layout prep. Returns per-core input maps."""
    e4 = ml_dtypes.float8_e4m3
    hidden_states = np.asarray(hidden_states)
    gate_w = np.asarray(gate_w)
    correction_bias = np.asarray(correction_bias)
    w_up = np.asarray(w_up)
    w_down = np.asarray(w_down)
    ws_up = np.asarray(ws_up)
    ws_down = np.asarray(ws_down)

    x = np.ascontiguousarray(hidden_states.astype(np.float32))
    xt = np.ascontiguousarray(x.T)                        # [H, T] f32
    x16 = xt.astype(np.float16)
    xr8g = np.ascontiguousarray(
        ((xt - x16.astype(np.float32)) * RS).astype(e4)
        .reshape(KT, 128, T).transpose(1, 0, 2)
    )
    gw32 = gate_w.astype(np.float32)                      # [E, H]
    cb32 = correction_bias.astype(np.float32)

    wu8_all = (w_up.astype(np.float32) * WSCALE).astype(e4)    # [E, H, I]
    wd8_all = (w_down.astype(np.float32) * WSCALE).astype(e4)  # [E, I, H]
    wsu_s = ws_up.astype(np.float32) * WSCALE
    wsd_s = ws_down.astype(np.float32) * WSCALE
    wsu8 = wsu_s.astype(e4)
    wsur8 = (wsu_s - wsu8.astype(np.float32)).astype(e4)
    wsd8 = wsd_s.astype(e4)
    wsdr8 = (wsd_s - wsd8.astype(np.float32)).astype(e4)

    in_maps = []
    for c in range(NCORES):
        # group-roll so this core's experts (= gate group c) sit in
        # gate columns 0..3; grouped top-k is order-invariant.
        roll = np.roll(np.arange(N_GROUP), -c)
        perm = (roll[:, None] * GSIZE + np.arange(GSIZE)).ravel()
        gperm = gw32[perm].T                               # [H, E] f32
        g16 = gperm.astype(np.float16)
        gr8 = ((gperm - g16.astype(np.float32)) * RS).astype(e4)

        def pk(a):
            return np.ascontiguousarray(
                a.reshape(KT, 128, E).transpose(1, 0, 2).reshape(128, KT * E)
            )

        biasb = np.broadcast_to(cb32[perm][None, :], (128, E)).copy()
        sl = slice(c * S_LOC, (c + 1) * S_LOC)
        in_maps.append(
            {
                "x16": x16,
                "xr8": xr8g,
                "g16": pk(g16),
                "gr8": pk(gr8),
                "biasb": biasb,
                "wu8": np.ascontiguousarray(wu8_all[c * E_LOC : (c + 1) * E_LOC]),
                "wd8": np.ascontiguousarray(wd8_all[c * E_LOC : (c + 1) * E_LOC]),
                "wsu8": np.ascontiguousarray(wsu8[:, sl]),
                "wsur8": np.ascontiguousarray(wsur8[:, sl]),
                "wsd8": np.ascontiguousarray(wsd8[sl, :]),
                "wsdr8": np.ascontiguousarray(wsdr8[sl, :]),
            }
        )
    return in_maps


_CACHED = {}


def _get_nc():
    if "nc" not in _CACHED:
        _CACHED["nc"] = _build_kernel()
    return _CACHED["nc"]


def kernel(hidden_states, gate_w, correction_bias, w_up, w_down, ws_up, ws_down):
    from concourse.bass_utils import run_bass_kernel_spmd

    nc = _get_nc()
    in_maps = _prep_inputs(
        hidden_states, gate_w, correction_bias, w_up, w_down, ws_up, ws_down
    )
    res = run_bass_kernel_spmd(nc, in_maps, list(range(NCORES)))
    out = np.zeros((T, H), np.float32)
    for r in res.results:
        out += np.asarray(r["out"]).astype(np.float32)
    return out
